# revision 6
# baseline (speedup 1.0000x reference)
"""Bass/Trainium2 kernel v2 for nn_BatchLoreAttentionLayer.

Per batch item b (X = embeddings[b], [L=128, D=256]):
    S = X A X^T + rowbias,  A = q_w^T k_w / sqrt(D)
    E = exp(S); g = valid/(rowsum*cnt); w = E^T g; out = tanh(X^T w)

v2 vs baseline (120852ns -> 92205ns cost-model): fp8 on the S chain.
 - xt (X^T view) ships fp8-e4m3 (1B/elem); xl (X view) ships bf16
   (V-side is precision-critical: fp8 xl alone costs 2.4e-2 rel err;
   measured HW rel err of this config: 1.36e-2 < 2e-2 gate).
 - A scaled x32 into fp8 (avoids e4m3 denormals; TRN e4m3 max is 240);
   exp un-scales via ACT scale=1/32. rowbias as e5m2 {-1024,0} stored
   [chunk, ktile, item, l] so the rank-1 DoubleRow bias matmul's rhs AP
   merges to [1, 2, N] with a <32767-elem step (16-bit ISA field).
 - Yt/S matmuls are fp8 DoubleRow (2 K-tiles/inst, 0.5 cyc/row = 4x
   bf16). One PSUM accumulation group per 2KB bank (start on the first
   item matmul only; the bias matmul stops) - overlapping starts in one
   zero region corrupt results on HW.
 - Yt = A'^T X^T on-chip; the PSUM->SBUF fp8 casts rotate ACT/DVE 11:9
   (GPSIMD has no PSUM port). Two-pass blocks: all Yt matmuls first so
   PE's in-order queue never stalls S behind a pending copy.
 - exp per group on ACT; rowsum reduce per 2 groups, recip/g-mul/w-copy
   batched per 4-group block on DVE (TensorReduce has no 2x DVE mode).
 - 4-group DMA tiles on the sync queue (HWDGE ~25us); DMA device 71.4us
   is the roofline (xl bf16 46.6 + xt fp8 23.3); ACT 77.6/DVE 75.7 busy.
 - vt/rb singles issue on the ACT HWDGE queue so block 0's xt load gets
   an early HWDGE slot on SP (-1.2us head). Out-DMA stays on sync: ACT/
   Pool-queue variants measured slower.

Sharding: pure data-parallel over B across 8 cores (256 items/core).
"""

import sys
from contextlib import ExitStack

import numpy as np
import ml_dtypes

sys.path.insert(0, "/opt/trn_rl_repo")

import concourse.bass as bass  # noqa: E402
import concourse.mybir as mybir  # noqa: E402
import concourse.tile as tile  # noqa: E402
from concourse import bacc  # noqa: E402
from concourse.bass import ts  # noqa: E402
from concourse.bass_utils import run_bass_kernel_spmd  # noqa: E402

B, L, D = 2048, 128, 256
NCORES = 8
BPC = B // NCORES   # items per core (256)
CHUNK = 128         # items per output chunk
GRP = 4             # items per group
BLK = 4             # groups per DMA block
GPB = GRP * BLK     # items per block (16)
ASCALE = 32.0       # host A pre-scale (fp8 denormal avoidance)

F32 = mybir.dt.float32
BF16 = mybir.dt.bfloat16
FP8 = mybir.dt.float8e4
FP8E5 = mybir.dt.float8e5
AF = mybir.ActivationFunctionType
DR = mybir.MatmulPerfMode.DoubleRow

# yt PSUM->SBUF copy rotation: ACT takes ROT_ACT of every ROT_PERIOD groups
ROT_PERIOD = 20
ROT_ACT = 11
# rowsum engine per exp-pair: Pool takes pairs with (pair % POOL_RS_MOD) < POOL_RS_K
POOL_RS_MOD = 2
POOL_RS_K = 0

_CACHE = {}


def build_bass():
    nc = bacc.Bacc(None, target_bir_lowering=False)
    nblk = BPC // GPB  # 16 DMA blocks per core
    xt = nc.declare_dram_parameter("xt", [nblk, 128, BLK * 2 * GRP * L], FP8, isOutput=False)
    xl = nc.declare_dram_parameter("xl", [nblk, 128, BLK * GRP * D], BF16, isOutput=False)
    rb = nc.declare_dram_parameter("rb", [BPC // CHUNK, 2, CHUNK, L], FP8E5, isOutput=False)
    vt = nc.declare_dram_parameter("vt", [L, BPC], F32, isOutput=False)
    aw = nc.declare_dram_parameter("aw", [128, 2 * D], FP8, isOutput=False)
    outT = nc.declare_dram_parameter("outT", [128, 2, BPC], F32, isOutput=True)
    build_body(nc, xt, xl, rb, vt, aw, outT)
    nc.finalize()
    return nc


def build_body(nc, xt, xl, rb, vt, aw, outT):
    rot = [(i * ROT_ACT) % ROT_PERIOD < ROT_ACT for i in range(ROT_PERIOD)]
    with tile.TileContext(nc) as tc, ExitStack() as ctx:
        singles = ctx.enter_context(tc.tile_pool(name="singles", bufs=1))
        io_xt = ctx.enter_context(tc.tile_pool(name="io_xt", bufs=6))
        io_xl = ctx.enter_context(tc.tile_pool(name="io_xl", bufs=6))
        wk_yt = ctx.enter_context(tc.tile_pool(name="wk_yt", bufs=6))
        wk_e = ctx.enter_context(tc.tile_pool(name="wk_e", bufs=6))
        small = ctx.enter_context(tc.tile_pool(name="small", bufs=16))
        wk_o = ctx.enter_context(tc.tile_pool(name="wk_o", bufs=2))
        wk_sc = ctx.enter_context(tc.tile_pool(name="wk_sc", bufs=2))
        ps_yt = ctx.enter_context(tc.tile_pool(name="ps_yt", bufs=2, space="PSUM"))
        ps_s = ctx.enter_context(tc.tile_pool(name="ps_s", bufs=3, space="PSUM"))
        ps_o = ctx.enter_context(tc.tile_pool(name="ps_o", bufs=1, space="PSUM"))

        # ---- one-time loads ----
        a_sb = singles.tile([128, 2, D], FP8)      # A'[dt*128+ds, e] = 32*A
        nc.sync.dma_start(out=a_sb, in_=aw.rearrange("p (t e) -> p t e", t=2))
        vt_sb = singles.tile([128, BPC], F32)      # valid/cnt, [l, b]
        nc.scalar.dma_start(out=vt_sb, in_=vt[:, :])
        # [chunk, k-tile, item, l]: k-tile stride 16K (fits 16-bit ISA step)
        # and (item, l) contiguous so the DR rhs AP merges to [1, 2, N].
        rb_sb = singles.tile([1, BPC // CHUNK, 2, CHUNK, L], FP8E5)  # on p0
        rb_r = rb.rearrange("c t (o b) l -> c o t b l", o=1)
        for i in range(BPC // CHUNK):
            nc.scalar.dma_start(out=rb_sb[:, i], in_=rb_r[i])
        ones_sb = singles.tile([1, 2, 128], FP8E5)
        nc.vector.memset(ones_sb, 1.0)

        n_chunks = BPC // CHUNK
        for c in range(n_chunks):
            # oT ([128, 2, 128] = 1KB) and wcol ([128, 128] = 512B) share one
            # PSUM bank (1.5KB < 2KB) so everything fits in 8 banks.
            ow_ps = ps_o.tile([128, 3, CHUNK], F32, tag="ow")
            oT_ps = ow_ps[:, 0:2]
            wcol_ps = ow_ps[:, 2]
            for blk in range(CHUNK // GPB):
                bi = c * (CHUNK // GPB) + blk
                xt_b = io_xt.tile([128, BLK, 2, GRP * L], FP8, tag="xt")
                xl_b = io_xl.tile([128, BLK, GRP, D], BF16, tag="xl")
                yt_blk = wk_yt.tile([128, BLK, 2, 512], FP8, tag="yt_blk")
                nc.sync.dma_start(
                    out=xt_b, in_=xt[bi].rearrange("p (g t n) -> p g t n", g=BLK, t=2)
                )
                nc.sync.dma_start(
                    out=xl_b, in_=xl[bi].rearrange("p (g j d) -> p g j d", g=BLK, j=GRP)
                )
                rs_b = small.tile([128, GPB], F32, tag="rs")
                # pass 1: all Yt matmuls + PSUM->SBUF casts (PE never waits)
                for gi in range(BLK):
                    gg = c * (CHUNK // GRP) + blk * BLK + gi
                    yt_ps = ps_yt.tile([128, 2, 512], F32, tag="yt")
                    for e2 in (0, 1):
                        nc.tensor.matmul(
                            out=yt_ps[:, e2],
                            lhsT=a_sb[:, :, ts(e2, 128)],
                            rhs=xt_b[:, gi],
                            start=True, stop=True, perf_mode=DR,
                        )
                    src, dst = yt_ps, yt_blk[:, gi]
                    if rot[gg % ROT_PERIOD]:
                        nc.scalar.activation(out=dst, in_=src, func=AF.Copy)
                    else:
                        nc.vector.tensor_copy(out=dst, in_=src)
                # pass 2: S (DoubleRow) + mask bias + exp + rowsum
                e4_tiles = []
                for gi in range(BLK):
                    g = blk * BLK + gi              # group within chunk
                    i0 = c * CHUNK + g * GRP        # item base within core
                    xt4 = xt_b[:, gi]               # [128, 2, 512]
                    yt_sb = yt_blk[:, gi]           # [128, 2, 512]
                    par = gi % 2
                    if par == 0:
                        e4x2 = wk_e.tile([128, 2, GRP, L], BF16, tag="e4x2")
                        e4_tiles.append(e4x2)
                    s4 = ps_s.tile([128, GRP, L], F32, tag="s4")
                    for j in range(GRP):
                        nc.tensor.matmul(
                            out=s4[:, j],
                            lhsT=yt_sb[:, :, ts(j, L)],
                            rhs=xt4[:, :, ts(j, L)],
                            start=(j == 0), stop=False, perf_mode=DR,
                            skip_group_check=True,
                        )
                    nc.tensor.matmul(
                        out=s4,
                        lhsT=ones_sb,
                        rhs=rb_sb[:, c, :, g * GRP : g * GRP + GRP, :],
                        start=False, stop=True, perf_mode=DR,
                        skip_group_check=True,
                    )
                    nc.scalar.activation(
                        out=e4x2[:, par], in_=s4, func=AF.Exp,
                        scale=1.0 / ASCALE,
                    )
                    if par == 1:
                        pair = (c * (CHUNK // GRP) + g) // 2
                        rs_out = rs_b[:, (gi - 1) * GRP : (gi + 1) * GRP]
                        if pair % POOL_RS_MOD < POOL_RS_K:
                            # rowsum via per-item cumsum scans on idle GPSIMD
                            sc = wk_sc.tile([128, 2, GRP, L], F32, tag="sc")
                            for a in range(2):
                                for j in range(GRP):
                                    nc.gpsimd.tensor_tensor_scan(
                                        out=sc[:, a, j],
                                        data0=e4x2[:, a, j],
                                        data1=e4x2[:, a, j],
                                        initial=0.0,
                                        op0=mybir.AluOpType.add,
                                        op1=mybir.AluOpType.bypass,
                                    )
                            nc.gpsimd.tensor_copy(
                                out=rs_out, in_=sc[:, :, :, L - 1]
                            )
                        else:
                            nc.vector.reduce_sum(
                                out=rs_out,
                                in_=e4x2.rearrange("p a j m -> p (a j) m"),
                                axis=mybir.AxisListType.X,
                            )
                # ---- per-block: g = vt/rowsum ; w = E^T g ; oT += X^T w ----
                i0b = c * CHUNK + blk * GPB
                col0 = blk * GPB
                rinv_b = small.tile([128, GPB], F32, tag="rinv")
                nc.vector.reciprocal(out=rinv_b, in_=rs_b)
                g_b = small.tile([128, GPB], BF16, tag="g")
                nc.vector.tensor_mul(g_b, rinv_b, vt_sb[:, i0b : i0b + GPB])
                for gi in range(BLK):
                    e4x2 = e4_tiles[gi // 2]
                    for j in range(GRP):
                        k = gi * GRP + j
                        nc.tensor.matmul(
                            out=wcol_ps[:, col0 + k : col0 + k + 1],
                            lhsT=e4x2[:, gi % 2, j],
                            rhs=g_b[:, k : k + 1],
                            start=True, stop=True,
                        )
                w_b = small.tile([128, GPB], BF16, tag="wb")
                nc.vector.tensor_copy(out=w_b, in_=wcol_ps[:, col0 : col0 + GPB])
                for gi in range(BLK):
                    for j in range(GRP):
                        k = gi * GRP + j
                        for dh in range(2):
                            nc.tensor.matmul(
                                out=oT_ps[:, dh, col0 + k : col0 + k + 1],
                                lhsT=xl_b[:, gi, j, ts(dh, 128)],
                                rhs=w_b[:, k : k + 1],
                                start=True, stop=True,
                            )
            # ---- tanh + store chunk ----
            oT_sb = wk_o.tile([128, 2, CHUNK], F32, tag="oT_sb")
            nc.scalar.activation(out=oT_sb, in_=oT_ps, func=AF.Tanh)
            nc.sync.dma_start(
                out=outT[:, :, c * CHUNK : (c + 1) * CHUNK], in_=oT_sb
            )


def prep_inputs(embeddings, padding_mask, q_w, q_b, k_w, k_b):
    emb = np.asarray(embeddings, np.float32)
    mask = np.asarray(padding_mask)
    q_w = np.asarray(q_w, np.float32)
    k_w = np.asarray(k_w, np.float32)
    q_b = np.asarray(q_b, np.float32)
    scale = 1.0 / np.sqrt(np.float32(D))

    f8 = ml_dtypes.float8_e4m3
    f8e5 = ml_dtypes.float8_e5m2
    bf = ml_dtypes.bfloat16

    A = (q_w.T @ k_w) * (scale * ASCALE)             # [D, D] (d, e), x32
    np.clip(A, -240.0, 240.0, out=A)
    # aw[ds, (dt e)] = A[dt*128+ds, e]
    aw = np.ascontiguousarray(
        A.reshape(2, 128, D).transpose(1, 0, 2).reshape(128, 2 * D)
    ).astype(f8)

    rowbias = np.where(mask, np.float32(-1024.0), np.float32(0.0))
    v = (k_w.T @ q_b) * scale
    if np.any(v):
        # general q_b path: fold s(m) term (halved across the two k-tiles)
        rowbias = rowbias + (emb @ v) * (ASCALE / 2.0)
    rb_host = rowbias.astype(f8e5)                   # [B, L]

    valid = (~mask).astype(np.float32)
    cnt = np.maximum(valid.sum(1, keepdims=True), 1.0)
    vt_full = np.ascontiguousarray((valid / cnt).T.astype(np.float32))  # [L, B]

    emb8 = emb.astype(f8)
    emb16 = emb.astype(bf)
    nblk_g = B // GPB
    # xt blocks: [nblk, 128(ds), (gi, dt, j, l)]
    xtg = (
        emb8.transpose(0, 2, 1)                       # [B, D, L]
        .reshape(nblk_g, BLK, GRP, 2, 128, L)         # [b4, gi, j, dt, ds, l]
        .transpose(0, 4, 1, 3, 2, 5)                  # [b4, ds, gi, dt, j, l]
        .reshape(nblk_g, 128, BLK * 2 * GRP * L)
    )
    # xl blocks: [nblk, 128(l), (gi, j, d)]
    xlg = (
        emb16.reshape(nblk_g, BLK, GRP, L, D)         # [b4, gi, j, l, d]
        .transpose(0, 3, 1, 2, 4)                     # [b4, l, gi, j, d]
        .reshape(nblk_g, 128, BLK * GRP * D)
    )
    xtg = np.ascontiguousarray(xtg)
    xlg = np.ascontiguousarray(xlg)

    npb = BPC // GPB
    in_maps = []
    for cidx in range(NCORES):
        sl = slice(cidx * BPC, (cidx + 1) * BPC)
        rbs = np.ascontiguousarray(rb_host[sl])       # [BPC, L]
        in_maps.append(
            {
                "xt": xtg[cidx * npb : (cidx + 1) * npb],
                "xl": xlg[cidx * npb : (cidx + 1) * npb],
                "rb": np.ascontiguousarray(
                    np.stack([rbs.reshape(BPC // CHUNK, CHUNK, L)] * 2, axis=1)
                ),
                "vt": np.ascontiguousarray(vt_full[:, sl]),
                "aw": aw,
            }
        )
    return in_maps


def _get_nc():
    if "nc" not in _CACHE:
        _CACHE["nc"] = build_bass()
    return _CACHE["nc"]


def _make_exec():
    """Build the shard_map'd PJRT executable once (see baseline kernel)."""
    import jax
    from jax.sharding import Mesh, PartitionSpec
    from jax.experimental.shard_map import shard_map
    from concourse import bass2jax, mybir as _mybir

    nc = _get_nc()
    bass2jax.install_neuronx_cc_hook()
    partition_name = nc.partition_id_tensor.name if nc.partition_id_tensor else None
    in_names, out_names, out_avals, zero_outs = [], [], [], []
    for alloc in nc.m.functions[0].allocations:
        if not isinstance(alloc, _mybir.MemoryLocationSet):
            continue
        name = alloc.memorylocations[0].name
        if alloc.kind == "ExternalInput":
            if name != partition_name:
                in_names.append(name)
        elif alloc.kind == "ExternalOutput":
            shape = tuple(alloc.tensor_shape)
            dtype = _mybir.dt.np(alloc.dtype)
            out_names.append(name)
            out_avals.append(jax.core.ShapedArray(shape, dtype))
            zero_outs.append(np.zeros(shape, dtype))
    n_params = len(in_names)
    in_names_full = in_names + out_names
    if partition_name is not None:
        in_names_full.append(partition_name)

    def _body(*args):
        operands = list(args)
        if partition_name is not None:
            operands.append(bass2jax.partition_id_tensor())
        outs = bass2jax._bass_exec_p.bind(
            *operands,
            out_avals=tuple(out_avals),
            in_names=tuple(in_names_full),
            out_names=tuple(out_names),
            lowering_input_output_aliases=(),
            sim_require_finite=True,
            sim_require_nnan=True,
            nc=nc,
        )
        return tuple(outs)

    devices = jax.devices()[:NCORES]
    mesh = Mesh(np.asarray(devices), ("core",))
    n_outs = len(out_names)
    sharded = jax.jit(
        shard_map(
            _body,
            mesh=mesh,
            in_specs=(PartitionSpec("core"),) * (n_params + n_outs),
            out_specs=(PartitionSpec("core"),) * n_outs,
            check_rep=False,
        ),
        donate_argnums=tuple(range(n_params, n_params + n_outs)),
        keep_unused=True,
    )

    def run(in_maps, n_iters=1, timings=None):
        import time as _t

        concat_in = [
            np.concatenate([np.asarray(in_maps[c][nm]) for c in range(NCORES)], axis=0)
            for nm in in_names
        ]
        placed = [jax.device_put(a) for a in concat_in]
        zo = [np.concatenate([z] * NCORES, axis=0) for z in zero_outs]
        outs = None
        for _ in range(n_iters):
            zplaced = [jax.device_put(z) for z in zo]
            for p in placed + zplaced:
                p.block_until_ready()
            t0 = _t.perf_counter()
            outs = sharded(*placed, *zplaced)
            for o in outs:
                o.block_until_ready()
            if timings is not None:
                timings.append(_t.perf_counter() - t0)
        res = []
        for c in range(NCORES):
            d = {}
            for i, nm in enumerate(out_names):
                full = np.asarray(outs[i])
                per = full.shape[0] // NCORES
                d[nm] = full[c * per : (c + 1) * per]
            res.append(d)
        return res

    return run


def _get_runner():
    if "run" not in _CACHE:
        _CACHE["run"] = _make_exec()
    return _CACHE["run"]


def kernel(embeddings, padding_mask, q_w, q_b, k_w, k_b, _n_iters=None, _timings=None):
    in_maps = prep_inputs(embeddings, padding_mask, q_w, q_b, k_w, k_b)
    if _n_iters is None:
        res = run_bass_kernel_spmd(_get_nc(), in_maps, list(range(NCORES)))
        results = res.results
    else:
        results = _get_runner()(in_maps, n_iters=_n_iters, timings=_timings)
    out = np.empty((B, D), np.float32)
    for c in range(NCORES):
        oT = np.asarray(results[c]["outT"], np.float32)  # [128, 2, BPC]
        out[c * BPC : (c + 1) * BPC] = oT.transpose(2, 1, 0).reshape(BPC, D)
    return out


if __name__ == "__main__":
    ref_inputs = {
        "embeddings": np.random.randn(B, L, D).astype(np.float32),
        "padding_mask": np.random.rand(B, L) < 0.3,
        "q_w": np.random.randn(D, D).astype(np.float32) * 0.06,
        "q_b": np.zeros(D, np.float32),
        "k_w": np.random.randn(D, D).astype(np.float32) * 0.06,
        "k_b": np.zeros(D, np.float32),
    }
    out = kernel(**ref_inputs)
    print(out.shape, out.dtype)
